# revision 67
# baseline (speedup 1.0000x reference)
r"""DetCon (NT-Xent style) contrastive loss on 8 Trainium2 NeuronCores.

Reference computes, for v0/v1 = L2-normalized (over E) views scaled by
1/sqrt(T):   logits = [[S01, S00\diag], [S10, S11\diag]]  (2BN x 2BN-1)
             loss = mean_i( logsumexp(row_i) - label_logit_i )
with label_logit_i = S01[i,i] (== S10[i,i]).

v3: TRANSPOSED tiles. Each core owns 1024 rows (512 per view); logits are
computed as S^T tiles [128 key-cols (partitions), 1024 rows (free)]:
  - the host pre-quantizes both views to fp8e4 and pre-transposes to
    [E, B*N], so the whole input is 4 contiguous DMAs (~2.1 MB) and the
    keys need NO on-chip copy or per-column normalize multiply at all
  - the key-side normalize scale s_j is per-PARTITION in this layout:
    folded into the exp for free (ACT `scale` arg / DVE Schraudolph
    per-partition scalar); the row-side scale s_r is folded into the
    fp8 moving operand (rows8, scaled via a broadcast sumsq + Ln/Exp)
  - row-sums of exp become PARTITION sums -> tiny PE ones-matmuls
    (fp8 DoubleRow for ACT tiles, bf16 for DVE Schraudolph tiles)
    accumulated into one [32, 1024] PSUM tile; no ACT accum-reads, no
    DVE second pass; the rowsum matmuls are emitted DELAY units behind
    the exps so PE's in-order queue can produce logits tiles ahead
  - key scales are computed COMPACTLY: sum-of-squares -> [1,512] PSUM
    rows (M=32 DoubleRow ones-matmul) -> SBUF -> PE-transpose ->
    [128, nn] -> ACT Ln+Exp ((0.1*ss)^-0.5), all inside the one
    activation table that also serves the main Exp and the final Ln --
    zero activation-table reloads, no DVE reciprocal, no Sqrt
  - same-view diagonal removed EXACTLY by zero-masking 128x128 diag
    blocks of the exp tiles (gpsimd); label logits extracted from PSUM
    via DVE stst+identity then scaled by sclT
Host sums the 8 per-core partial sums (+5 fold-back) and divides by 2BN.
"""

from contextlib import ExitStack

import numpy as np

import concourse.bacc as bacc
import concourse.tile as tile
from concourse import mybir
from concourse.bass_utils import run_bass_kernel_spmd
from concourse.hw_specs import get_activation_tables

B, E, N = 64, 256, 64
BN = B * N            # 4096 columns per view
NCORES = 8
CHUNK = BN // NCORES  # 512 rows (of each view) per core
ROWS = 2 * CHUNK      # 1024 moving rows per core
P = 128
KH = E // P           # 2 contraction halves
LG = 8                # load groups (512 columns each)
GL = BN // LG         # 512
GB = B // LG          # 8 b-slices per load group
EXPB = -5.0           # exp bias: tiles hold e^(l-5) so fp8 never overflows

# bf16 Schraudolph exp: bits(e^x) ~= int16(x * 184.66 + (16256 + C))
SCH_A = 184.6649652337873
SCH_C = -5.0
SCH_B = 16256.0 + SCH_C + EXPB * SCH_A

F32 = mybir.dt.float32
BF16 = mybir.dt.bfloat16
FP8 = mybir.dt.float8e4
I16 = mybir.dt.int16
AF = mybir.ActivationFunctionType

# ACT share of the 32 jb pairs
ACT_PAIRS = 18
# scale-group batching: early lgs fine-grained (their scales gate early
# exps), late lgs coarse (their exps run late anyway)
SG_LGS = ((0,), (1,), (2, 3), (4, 5, 6, 7))


def _cc(v, lg, q):
    """Compact scale column for view v, load-group lg, 128-chunk q."""
    return lg * 8 + v * 4 + q


def _build_schedule():
    """Work units for the main phase: jb pairs assigned to ACT (fp8 exp,
    shared DoubleRow ones-matmul) or DVE (Schraudolph bf16), interleaved
    by estimated finish time. Both queues sweep jc in order."""
    pairs = [(i, kv) for i in range(16) for kv in range(2)]
    asg = {}
    qa = qv = 0
    va, vv = ACT_PAIRS, 32 - ACT_PAIRS
    for p in pairs:
        if vv * qa <= va * qv:
            asg[p] = 'A'
            qa += 1
        else:
            asg[p] = 'V'
            qv += 1
    ua = [(kv, (2 * i, 2 * i + 1)) for (i, kv) in pairs if asg[(i, kv)] == 'A']
    uv = [(kv, (2 * i, 2 * i + 1)) for (i, kv) in pairs if asg[(i, kv)] == 'V']
    out = []
    ta = tv = 0.0
    ia = iv = 0
    TA, TV = 2.1, 2.7
    while ia < len(ua) or iv < len(uv):
        if iv >= len(uv) or (ia < len(ua) and ta + TA <= tv + TV):
            out.append(('A',) + ua[ia])
            ia += 1
            ta += TA
        else:
            out.append(('V',) + uv[iv])
            iv += 1
            tv += TV
    return out


SCHEDULE = _build_schedule()


def _emit_loads(nc, pl, vin, r, raw):
    """4 fully-contiguous fp8 DMAs (host pre-quantized + pre-transposed
    to [E, B*N]): view0 on SP, view1 on ACT -> all data lands ~3us."""
    for v in range(2):
        for h in range(KH):
            eng = nc.scalar if v == 1 else nc.sync
            eng.dma_start(out=raw[v][:, h, :],
                          in_=vin[v][h * P:(h + 1) * P, :])


_LG_SG = {lg: (sg, i) for sg, lgs in enumerate(SG_LGS)
          for i, lg in enumerate(lgs)}


def _emit_norm_lg(nc, pl, r, v, lg, raw, keys8, ssc_sb, bc_out=None):
    """Per (view, load-group): squares, sumsq, compact ss copy.
    For lg0 also emits a broadcast sumsq (M=128) used by the rows8 path."""
    ones8_1 = pl["consts"]["ones8_1"]
    gs = slice(lg * GL, (lg + 1) * GL)
    sq = pl["sq"].tile([P, KH, GL], FP8, tag="sq", name=f"sq{v}{lg}_{r}")
    for h in range(KH):
        nc.gpsimd.tensor_mul(sq[:, h, :], raw[v][:, h, gs], raw[v][:, h, gs])
    if bc_out is not None:
        nc.tensor.matmul(bc_out, pl["consts"]["ones8F"][:], sq[:, :, :],
                         perf_mode=mybir.MatmulPerfMode.DoubleRow)
    ssc = pl["pt"].tile([32, GL], F32, tag="pt", name=f"ssc{v}{lg}_{r}")
    nc.tensor.matmul(ssc[:], ones8_1[:], sq[:, :, :],
                     perf_mode=mybir.MatmulPerfMode.DoubleRow)
    t = pl["sml"].tile([1, GL], F32, tag=f"sscb{v}{lg}", name=f"sscb{v}{lg}_{r}")
    if lg >= 6:
        # late groups: ACT has idle there, DVE is the critical engine
        nc.scalar.activation(t[:], ssc[0:1, :], AF.Copy)
    else:
        nc.vector.tensor_copy(t[:], ssc[0:1, :])
    ssc_sb[(v, lg)] = t


def _emit_scale_sg(nc, pl, r, sg, ssc_sb, sclT, sAT):
    """Transpose compact ss chunks and compute sclT = (0.1*ss)^-0.5."""
    ident = pl["consts"]["ident"]
    lgs = SG_LGS[sg]
    nn = 2 * 4 * len(lgs)
    pssT = pl["pt"].tile([P, nn], F32, tag="pt", name=f"pssT{sg}_{r}")
    cc0 = _cc(0, lgs[0], 0)
    for v in range(2):
        for lg in lgs:
            for q in range(4):
                col = _cc(v, lg, q) - cc0
                nc.tensor.transpose(
                    pssT[:, col:col + 1],
                    ssc_sb[(v, lg)][0:1, q * P:(q + 1) * P],
                    ident[0:1, 0:1])
    lnt = pl["sml"].tile([P, nn], F32, tag=f"lnt{sg}", name=f"lnt{sg}_{r}")
    nc.scalar.activation(lnt[:], pssT[:], AF.Ln, scale=0.1)
    nc.scalar.activation(sclT[:, cc0:cc0 + nn], lnt[:], AF.Exp, scale=-0.5)
    nc.vector.tensor_scalar(sAT[:, cc0:cc0 + nn], sclT[:, cc0:cc0 + nn],
                            SCH_A, 0.0, op0=mybir.AluOpType.mult,
                            op1=mybir.AluOpType.add)


def _emit_rows8(nc, pl, r, raw, ssb_bc, rows8):
    """rows8[:, h, v*512:(v+1)*512] = raw_rows * s_row (fp8). The row
    scale comes straight from the broadcast sumsq via ACT Ln+Exp --
    no transpose/copy/broadcast detour on the critical path."""
    lnb = pl["sml"].tile([P, ROWS], F32, tag="lnb", name=f"lnb_{r}")
    nc.scalar.activation(lnb[:], ssb_bc[:], AF.Ln, scale=0.1)
    sclb = pl["sml"].tile([P, ROWS], BF16, tag="sclb", name=f"sclb_{r}")
    nc.scalar.activation(sclb[:], lnb[:], AF.Exp, scale=-0.5)
    for v in range(2):
        for h in range(KH):
            nc.gpsimd.tensor_mul(rows8[:, h, v * CHUNK:(v + 1) * CHUNK],
                                 raw[v][:, h, 0:CHUNK],
                                 sclb[:, v * CHUNK:(v + 1) * CHUNK])


def _emit_unit(nc, pl, r, u, eng, kv, jcs, keys8, rows8, sclT, sAT, diag01):
    """Produce phase of one work unit: logits, (label/mask), exp.
    Returns the reduce closure (rowsum matmuls), emitted DELAYed so PE's
    in-order queue can run tiles ahead of exp completion."""
    ident = pl["consts"]["ident"]
    bias5 = pl["consts"]["bias5"]
    m8 = pl["consts"]["m8"]
    if eng == 'A':
        esc = pl["esc"].tile([P, 2, ROWS], FP8, tag="esc", name=f"esc{u}_{r}")
        ebfs = None
    else:
        ebfs = []
    for s, jc in enumerate(jcs):
        lg, q = jc // 4, jc % 4
        cc = _cc(kv, lg, q)
        pt = pl["pt"].tile([P, ROWS], F32, tag="pt", name=f"pt{u}{s}_{r}")
        lhsT = keys8[kv][:, :, jc * P:(jc + 1) * P]
        nc.tensor.matmul(pt[:, 0:CHUNK], lhsT, rows8[:, :, 0:CHUNK],
                         perf_mode=mybir.MatmulPerfMode.DoubleRow)
        nc.tensor.matmul(pt[:, CHUNK:ROWS], lhsT, rows8[:, :, CHUNK:ROWS],
                         perf_mode=mybir.MatmulPerfMode.DoubleRow)
        if lg == 0:
            # label logit: cross-view diag, rows of view (1-kv)
            dt = kv * 4 + jc
            dsc = pl["dsc"].tile([P, P], BF16, tag="dsc", name=f"dsc{u}{s}_{r}")
            nc.vector.scalar_tensor_tensor(
                dsc[:], pt[:, (1 - kv) * CHUNK + jc * P:
                            (1 - kv) * CHUNK + (jc + 1) * P],
                1.0, ident[:],
                op0=mybir.AluOpType.mult, op1=mybir.AluOpType.mult,
                accum_out=diag01[:, dt:dt + 1])
        if eng == 'A':
            nc.scalar.activation(esc[:, s, :], pt[:], AF.Exp,
                                 scale=sclT[:, cc:cc + 1], bias=bias5[:])
            if lg == 0:
                ds = kv * CHUNK + jc * P
                nc.gpsimd.tensor_mul(esc[:, s, ds:ds + P],
                                     esc[:, s, ds:ds + P], m8[:])
        else:
            it = pl["i16"].tile([P, ROWS], I16, tag="i16", name=f"it{u}{s}_{r}")
            nc.vector.tensor_scalar(
                it[:], pt[:], sAT[:, cc:cc + 1], SCH_B,
                op0=mybir.AluOpType.mult, op1=mybir.AluOpType.add)
            if lg == 0:
                ds = kv * CHUNK + jc * P
                nc.gpsimd.tensor_mul(it[:, ds:ds + P].bitcast(BF16),
                                     it[:, ds:ds + P].bitcast(BF16), m8[:])
            ebfs.append(it[:].bitcast(BF16))

    def reduce(rsA, rsB, first, last):
        ones8_1 = pl["consts"]["ones8_1"]
        ones1b = pl["consts"]["ones1b"]
        if eng == 'A':
            nc.tensor.matmul(rsA, ones8_1[:], esc[:, :, 0:CHUNK],
                             perf_mode=mybir.MatmulPerfMode.DoubleRow,
                             start=first, stop=last)
            nc.tensor.matmul(rsB, ones8_1[:], esc[:, :, CHUNK:ROWS],
                             perf_mode=mybir.MatmulPerfMode.DoubleRow,
                             start=first, stop=last)
        else:
            for k, ebf in enumerate(ebfs):
                f = first and k == 0
                l = last and k == len(ebfs) - 1
                nc.tensor.matmul(rsA, ones1b[:], ebf[:, 0:CHUNK],
                                 start=f, stop=l)
                nc.tensor.matmul(rsB, ones1b[:], ebf[:, CHUNK:ROWS],
                                 start=f, stop=l)
    return reduce


def _emit_epilogue(nc, pl, out_dram, r, rs, diag01, sclT):
    ones_col = pl["consts"]["ones_col"]
    lnr = pl["sml"].tile([1, ROWS], F32, tag="lnr", name=f"lnr{r}")
    lns = pl["sml"].tile([1, 1], F32, tag="lns", name=f"lns{r}")
    nc.scalar.activation(lnr[:], rs[0:1, :], AF.Ln, accum_out=lns[:])
    lab = pl["sml"].tile([P, 8], F32, tag="lab", name=f"lab{r}")
    nc.vector.tensor_mul(lab[:], diag01[:], sclT[:, 0:8])
    dsum = pl["sml"].tile([P, 1], F32, tag="dsum", name=f"dsum{r}")
    nc.vector.tensor_reduce(dsum[:], lab[:], axis=mybir.AxisListType.X,
                            op=mybir.AluOpType.add)
    fp = pl["pt"].tile([1, GL], F32, tag="pt", name=f"fp{r}")
    nc.tensor.matmul(fp[0:1, 0:1], dsum[:], ones_col[:])
    res = pl["sml"].tile([1, 1], F32, tag="res", name=f"res{r}")
    nc.vector.tensor_sub(res[:], lns[:], fp[0:1, 0:1])
    nc.sync.dma_start(out=out_dram[:], in_=res[:])


def _emit_pass(nc, pl, vin, out_dram, r, do_setup=True, do_main=True,
               state_prev=None):
    """One full loss computation (rep r)."""
    if do_setup:
        raw = [pl["raw"].tile([P, KH, BN], FP8, tag=f"raw{v}",
                              name=f"raw{v}_{r}") for v in range(2)]
        keys8 = raw
        ssc_sb = {}
        sclT = pl["sml"].tile([P, 64], F32, tag="sclT", name=f"sclT{r}")
        sAT = pl["sml"].tile([P, 64], F32, tag="sAT", name=f"sAT{r}")
        rows8 = pl["nrm"].tile([P, KH, ROWS], FP8, tag="rows8",
                               name=f"rows8_{r}")
        _emit_loads(nc, pl, vin, r, raw)
        if r == 0:
            # preload the one activation table that serves both Exp and Ln
            # (after the view1 DMAs so they hit the ACT queue first)
            tables = list(get_activation_tables(nc.m.arch).items())
            tidx = next(i for i, (nm, _) in enumerate(tables)
                        if nm == "natural_log_exp_and_others")
            nc.scalar.add_instruction(mybir.InstLoadActFuncSet(
                name=nc.get_next_instruction_name(), ins=[], outs=[],
                act_func_set_id=tidx))
        # group 0 first: normalize, rows8 (gates all logits), scale sg0
        ssb_bc = pl["pt"].tile([P, ROWS], F32, tag="pt", name=f"ssbc_{r}")
        for v in range(2):
            _emit_norm_lg(nc, pl, r, v, 0, raw, keys8, ssc_sb,
                          bc_out=ssb_bc[:, v * CHUNK:(v + 1) * CHUNK])
        _emit_rows8(nc, pl, r, raw, ssb_bc, rows8)
        _emit_scale_sg(nc, pl, r, 0, ssc_sb, sclT, sAT)
        state = (keys8, rows8, sclT, sAT, ssc_sb, raw)
    else:
        state = state_prev
        keys8, rows8, sclT, sAT, ssc_sb, raw = state

    def _advance_setup(done_lg, want_lg):
        while done_lg < want_lg:
            done_lg += 1
            for v in range(2):
                _emit_norm_lg(nc, pl, r, v, done_lg, raw, keys8, ssc_sb)
            sg, _ = _LG_SG[done_lg]
            if done_lg == SG_LGS[sg][-1]:
                _emit_scale_sg(nc, pl, r, sg, ssc_sb, sclT, sAT)
        return done_lg

    if do_main:
        rs = pl["rs"].tile([32, ROWS], F32, tag="rs", name=f"rs{r}")
        rsA = rs[:, 0:CHUNK]
        rsB = rs[:, CHUNK:ROWS]
        diag01 = pl["sml"].tile([P, 8], F32, tag="diag01", name=f"diag01{r}")
        done_lg = 0
        DELAY = 3  # units between exp emission and its rowsum matmuls
        pending = []
        nred = 0
        for u, (eng, kv, jcs) in enumerate(SCHEDULE):
            if do_setup:
                # emit normalize one load-group ahead of consumption (and
                # through the end of the scale group that covers this unit)
                mx = max(jc // 4 for jc in jcs)
                want = max(SG_LGS[_LG_SG[mx][0]][-1], min(mx + 1, LG - 1))
                done_lg = _advance_setup(done_lg, want)
            pending.append(_emit_unit(nc, pl, r, u, eng, kv, jcs, keys8,
                                      rows8, sclT, sAT, diag01))
            if len(pending) > DELAY:
                pending.pop(0)(rsA, rsB, nred == 0, False)
                nred += 1
        for i, red in enumerate(pending):
            red(rsA, rsB, nred == 0, i == len(pending) - 1)
            nred += 1
        _emit_epilogue(nc, pl, out_dram, r, rs, diag01, sclT)
    elif do_setup:
        _advance_setup(0, LG - 1)
    return state


def _build_nc(reps: int = 1, mode: str = "full"):
    """mode: 'full' reps everything; 'main' reps only logits+exp (one
    shared setup); 'setup' reps only load+normalize."""
    nc = bacc.Bacc()
    vin = [
        nc.dram_tensor("view0", [E, BN], FP8, kind="ExternalInput"),
        nc.dram_tensor("view1", [E, BN], FP8, kind="ExternalInput"),
    ]
    ident_in = nc.dram_tensor("ident", [P, P], F32, kind="ExternalInput")
    out_dram = nc.dram_tensor("out", [1, 1], F32, kind="ExternalOutput")

    with ExitStack() as ctx:
        tc = ctx.enter_context(tile.TileContext(nc))
        pl = {
            name: ctx.enter_context(tc.tile_pool(name=name, bufs=bufs))
            for name, bufs in (("raw", 1), ("sq", 2), ("nrm", 1),
                               ("esc", 4), ("i16", 6), ("dsc", 2),
                               ("sml", 1))
        }
        pl["pt"] = ctx.enter_context(
            tc.tile_pool(name="pt", bufs=3, space="PSUM"))
        pl["rs"] = ctx.enter_context(
            tc.tile_pool(name="rs", bufs=1, space="PSUM"))

        ident = pl["sml"].tile([P, P], F32, tag="ident", name="ident")
        nc.sync.dma_start(out=ident[:], in_=ident_in[:])
        consts = {"ident": ident}
        consts["ones8_1"] = pl["sml"].tile([P, KH, 32], FP8, tag="ones8_1",
                                           name="ones8_1")
        nc.vector.memset(consts["ones8_1"][:], 1.0)
        consts["ones8F"] = pl["sml"].tile([P, KH, P], FP8, tag="ones8F",
                                          name="ones8F")
        nc.vector.memset(consts["ones8F"][:], 1.0)
        consts["ones1b"] = pl["sml"].tile([P, 32], BF16, tag="ones1b",
                                          name="ones1b")
        nc.vector.memset(consts["ones1b"][:], 1.0)
        consts["onesP"] = pl["sml"].tile([1, P], BF16, tag="onesP",
                                         name="onesP")
        nc.vector.memset(consts["onesP"][:], 1.0)
        consts["ones_col"] = pl["sml"].tile([P, 1], F32, tag="ones_col",
                                            name="ones_col")
        nc.vector.memset(consts["ones_col"][:], 1.0)
        consts["bias5"] = pl["sml"].tile([P, 1], F32, tag="bias5",
                                         name="bias5")
        nc.vector.memset(consts["bias5"][:], EXPB)
        m8 = pl["sml"].tile([P, P], BF16, tag="m8", name="m8")
        nc.vector.memset(m8[:], 1.0)
        nc.vector.tensor_sub(m8[:], m8[:], ident[:])
        consts["m8"] = m8
        pl["consts"] = consts

        state = None
        for r in range(reps):
            state = _emit_pass(
                nc, pl, vin, out_dram, r,
                do_setup=(mode != "main" or r == 0),
                do_main=(mode != "setup"),
                state_prev=state)

    nc.compile()
    return nc


_NC_CACHE = None


def prep_in_maps(view0: np.ndarray, view1: np.ndarray):
    """Host-side shard prep: fp8-quantize, transpose to [E, B*N], and
    rotate so each core's own 512 rows come first."""
    fp8 = mybir.dt.np(FP8)
    ident = np.eye(P, dtype=np.float32)
    ems = [np.ascontiguousarray(v.transpose(1, 0, 2).reshape(E, BN))
           .astype(fp8) for v in (view0, view1)]
    in_maps = []
    for c in range(NCORES):
        sh = -c * CHUNK
        in_maps.append({
            "view0": np.ascontiguousarray(np.roll(ems[0], sh, axis=1)),
            "view1": np.ascontiguousarray(np.roll(ems[1], sh, axis=1)),
            "ident": ident,
        })
    return in_maps


def _run_spmd(view0: np.ndarray, view1: np.ndarray, nc=None, **spmd_kwargs):
    global _NC_CACHE
    if nc is None:
        if _NC_CACHE is None:
            _NC_CACHE = _build_nc()
        nc = _NC_CACHE

    in_maps = prep_in_maps(view0, view1)
    res = run_bass_kernel_spmd(nc, in_maps, core_ids=list(range(NCORES)),
                               **spmd_kwargs)
    total = sum(float(r["out"][0, 0]) for r in res.results)
    # every nll term carries a +5 from the e^(l-5) tiles
    return np.float32(total / (2 * BN) - EXPB)


def kernel(view0: np.ndarray, view1: np.ndarray) -> np.ndarray:
    return _run_spmd(view0, view1)
